# revision 8
# baseline (speedup 1.0000x reference)
"""Trainium2 Bass kernel for nn_DepthGlobalPool (histogram_binning), v2.

Math: out[b,:,h,w] = means[bin(b,h,w)] where
  bin   = histogram bin of depth over global [min,max], 10 equal bins
  means = per-bin mean of (W @ features[p] + bias) over pixels p in the bin.
Because the 1x1 conv is linear, means = W @ (per-bin feature means) + bias,
so the conv itself runs on the HOST over the tiny [10, Cin] bin-sums and the
device only ever computes per-bin SUMS of raw features (phase A) and the
per-pixel scatter of the final means (phase B).

Distribution: data-parallel over batch B (2 batches per core on 8 cores);
the [10,128] per-core partials are reduced on host between the two NEFFs.

Phase A (per core): features arrive HOST-pre-transposed as fp16 [128px, C]
blocks (fp16 keeps the HBM read at half of f32; quantization noise averages
out over ~7M pixels per bin). Per 128-px block j one accumulating matmul
  S_ps[10, 128] += onehot_j[128px, 10].T @ ftT_j[128px, 128c]
with the one-hot as the 10-column stationary (LDWEIGHTS ~8ns, hidden) and
the feature block streamed (128 cols). The one-hot itself is built on
device from a [128, 576] fp16 bin-id map (147 KB DMA instead of 2.4 MB).

Phase B (per core): out tile [128=(half,chan), 512px] = mstat.T @ onehot
per 512-px chunk, ONE K=40 matmul per chunk: the stationary is
block-diagonal [40,128] fp16 with means at rows 10g / col block 64i for
quarter g = 2b+i; the rhs packs the 4 quarters' one-hot at rows 10g.
Output is written fp16 (means are fp16-exact through f32 PSUM) and upcast
to f32 on host during the unshard -- halves the dominant write stream.
"""

import os
import numpy as np

import concourse.bass as bass  # noqa: F401  (registers types)
import concourse.tile as tile
import concourse.bass_utils as bass_utils
from concourse import bacc, mybir

# Problem shape (hardcoded per contract)
B, CIN, COUT, H, W_ = 16, 128, 64, 192, 192
HW = H * W_                      # 36864
NB = 10                          # histogram bins
N_CORES = 8
BPC = B // N_CORES               # batches per core = 2
PPC = BPC * HW                   # pixels per core = 73728
BLK = 128                        # pixels per block
N_BLOCKS = PPC // BLK            # 576
SLAB_PX = 9216                   # pixels per feature DMA slab (2.36 MB fp16)
N_SLABS = PPC // SLAB_PX         # 8
BLK_PER_SLAB = SLAB_PX // BLK    # 72
OHA_STRIDE = 16                  # onehot block stride (pad 10->16: 32B align)

F16 = mybir.dt.float16
F32 = mybir.dt.float32

_CACHE = {}

# exec times (ns) of the last kernel() call, per NEFF, when tracing enabled
LAST_EXEC_NS = {}
LAST_RES = {}


def _install_ntff_hook():
    """Optionally enable NTFF profiling under axon (agent image lacks
    antenv.axon_hooks). Best-effort; harmless if unavailable."""
    import sys, types
    if "antenv.axon_hooks" in sys.modules:
        return True
    try:
        mod = types.ModuleType("antenv.axon_hooks")
        _hook = [None]
        mod.set_axon_ntff_profile_hook = lambda h: _hook.__setitem__(0, h)
        mod.get_axon_ntff_profile_hook = lambda: _hook[0]
        import antenv
        from trn_agent_boot.trn_boot import _ntff_profile_via_ctypes
        antenv.axon_hooks = mod
        sys.modules["antenv.axon_hooks"] = mod
        mod.set_axon_ntff_profile_hook(
            _ntff_profile_via_ctypes("/opt/axon/libaxon_pjrt.so"))
        return True
    except Exception:
        return False


def _build_neff_a():
    """Phase A: per-core per-bin sums of (transposed) features, S[10, 128]."""
    nc = bacc.Bacc("TRN2", target_bir_lowering=False, debug=False,
                   enable_asserts=True, num_devices=N_CORES)
    ft_t = nc.dram_tensor("ft", [128, PPC], F16, kind="ExternalInput")
    bins_t = nc.dram_tensor("binsT", [128, N_BLOCKS], F16, kind="ExternalInput")
    spart_t = nc.dram_tensor("spart", [NB, CIN], F32, kind="ExternalOutput")

    ft = ft_t.ap()
    with tile.TileContext(nc) as tc:
        with tc.tile_pool(name="cst", bufs=1) as cst, \
             tc.tile_pool(name="fpool", bufs=4) as fpool, \
             tc.tile_pool(name="spool", bufs=1) as spool, \
             tc.tile_pool(name="pwarm", bufs=1, space="PSUM") as pwarm, \
             tc.tile_pool(name="pacc", bufs=1, space="PSUM") as pacc:

            binsT = cst.tile([128, N_BLOCKS], F16)
            nc.scalar.dma_start(binsT[:], bins_t.ap()[:])

            # dependency-free warmup burst: dense matmuls trip the PE HAM
            # clock-gate to 2.4 GHz while the first DMAs land; long enough
            # that the PE never sees a >3.4us idle gap before real work
            warm = cst.tile([128, 512], F16)
            nc.gpsimd.memset(warm[:], 0)
            wps = pwarm.tile([128, 512], F32, space="PSUM")
            for _ in range(20):
                nc.tensor.matmul(wps[:], lhsT=warm[:, :128], rhs=warm[:],
                                 start=True, stop=True)

            # one-hot oha[p, j, n] = (binsT[p, j] == n), built on device.
            # Two column-chunks so the first slabs' matmuls don't wait on
            # the whole [128, 576] sweep.
            oha = cst.tile([128, N_BLOCKS, OHA_STRIDE], F16)
            CHUNK = [(0, 3 * BLK_PER_SLAB), (3 * BLK_PER_SLAB, N_BLOCKS)]
            for j0, j1 in CHUNK:
                for n in range(NB):
                    nc.vector.tensor_scalar(
                        oha[:, j0:j1, n], binsT[:, j0:j1], float(n), None,
                        op0=mybir.AluOpType.is_equal)

            # 4-way column-tiled accumulators: block j accumulates into PE
            # column strip 32*(j%4), so 4 matmuls are in flight in the
            # array at once (the 10-col stationaries only need a strip)
            S_ps = pacc.tile([128, CIN], F32, space="PSUM")
            HALF = SLAB_PX // 2
            for s in range(N_SLABS):
                fs = fpool.tile([128, SLAB_PX], F16)
                # each slab's two 1.18MB halves ride DIFFERENT HWDGE rings
                # (sync + scalar): slab arrival order == program order even
                # when the rings drift, so the FIFO PE never waits on an
                # out-of-order slab while a later one sits ready
                nc.sync.dma_start(fs[:, 0:HALF], ft[:, s * SLAB_PX:
                                                     s * SLAB_PX + HALF])
                nc.scalar.dma_start(fs[:, HALF:], ft[:, s * SLAB_PX + HALF:
                                                     (s + 1) * SLAB_PX])
                for j in range(BLK_PER_SLAB):
                    blk = s * BLK_PER_SLAB + j
                    q = 32 * (blk % 4)
                    nc.tensor.matmul(
                        S_ps[q:q + NB, :],
                        lhsT=oha[:, blk, 0:NB],
                        rhs=fs[:, j * BLK:(j + 1) * BLK],
                        start=(blk < 4), stop=(blk >= N_BLOCKS - 4),
                        tile_position=(0, q))

            # PSUM-reduce the 4 strips (ops may read at most ONE PSUM input)
            s_acc = spool.tile([NB, CIN], F32)
            nc.vector.tensor_copy(s_acc[:], S_ps[0:NB, :])
            for q in (32, 64, 96):
                nc.vector.tensor_add(s_acc[:], s_acc[:], S_ps[q:q + NB, :])
            nc.sync.dma_start(spart_t.ap()[:], s_acc[:])
    nc.compile()
    return nc


def _build_neff_b():
    """Phase B: out[b,:,p] = means[bin(p)] via a means-stationary matmul.

    One K=40 matmul per [128, 512] chunk: stationary mst_b [40,128] fp16
    holds the fp16 means at rows 10g..10g+10 / col block 64i for quarter
    g = 2b+i; rhs oh_s packs quarter g's one-hot at rows 10g..10g+10
    (quarters for the OTHER batch hit zero stationary rows).

    Output staged in SBUF as fp16 [128=(half,chan), 4608] per slab, written
    in 2/2/2/3-chunk pieces so the write stream starts early; the
    half-interleaved [BPC, 128, HW/2] layout keeps every write a uniform
    2-D DMA (host undoes the interleave during the unshard).
    """
    nc = bacc.Bacc("TRN2", target_bir_lowering=False, debug=False,
                   enable_asserts=True, num_devices=N_CORES)
    mst_t = nc.dram_tensor("mst", [BPC, 80, 128], F16, kind="ExternalInput")
    ohb_t = nc.dram_tensor("ohb", [8 * NB, PPC // 4], F16, kind="ExternalInput")
    HW2 = HW // 2
    out_t = nc.dram_tensor("out", [BPC, 128, HW2], F16, kind="ExternalOutput")

    SLAB = 4608                  # p2-columns per slab
    N_CH = SLAB // 512           # 9 psum chunks per slab

    out_ap = out_t.ap()
    ohb = ohb_t.ap()
    with tile.TileContext(nc) as tc:
        with tc.tile_pool(name="cst", bufs=1) as cst, \
             tc.tile_pool(name="stage", bufs=8) as stage, \
             tc.tile_pool(name="pwarm", bufs=1, space="PSUM") as pwarm, \
             tc.tile_pool(name="pout", bufs=7, space="PSUM") as pout:

            mst_s = cst.tile([80, BPC * 128], F16)
            for b in range(BPC):
                nc.sync.dma_start(mst_s[:, 128 * b:128 * (b + 1)],
                                  mst_t.ap()[b])

            # whole one-hot in ONE persistent tile, 2x row-replicated to 80
            # partitions: a [40, *] transfer only reaches ~5 of 16 SDMA
            # engines (~110 GB/s); [80, *] doubles the bytes but reaches
            # ~10 engines (~270 GB/s), so the load finishes sooner and the
            # chunk matmuls stop being paced by it. The stationary's rows
            # 40-79 are zero, so the duplicate rows contribute nothing.
            ohq = cst.tile([80, PPC // 4], F16)
            QD = (PPC // 4) // 8
            for q in range(8):
                nc.sync.dma_start(ohq[:, q * QD:(q + 1) * QD],
                                  ohb[:, q * QD:(q + 1) * QD])

            # warmup burst for the PE HAM clock-gate: long enough to bridge
            # the one-hot/means input latency so real chunks start warm
            warm = cst.tile([128, 512], F16)
            nc.gpsimd.memset(warm[:], 0)
            # prime the SWDGE write path so the first real output write
            # doesn't pay the descriptor-ring bootstrap
            scr_t = nc.dram_tensor("scratch", [128, 64], F16, kind="Internal")
            nc.gpsimd.dma_start(scr_t.ap()[:], warm[:, 0:64])
            wps = pwarm.tile([128, 512], F32, space="PSUM")
            for _ in range(16):
                nc.tensor.matmul(wps[:], lhsT=warm[:, :128], rhs=warm[:],
                                 start=True, stop=True)

            ci = 0
            for cs in range(4):      # one-hot column slab: cols [o2, o2+4608)
                o2 = cs * SLAB
                for b in range(BPC):
                    if cs or b:
                        # filler matmuls at group boundaries: if the next
                        # group's chunks stall on the one-hot load, these
                        # keep the PE HAM window busy so the real matmuls
                        # run at 2.4 GHz instead of re-warming from 1.2
                        for _ in range(2):
                            nc.tensor.matmul(wps[:, :256], lhsT=warm[:, :128],
                                             rhs=warm[:, :256],
                                             start=True, stop=True)
                    # 9 chunks staged as pieces of 4/5 chunks: big pieces
                    # keep the SWDGE descriptor lines fat (>=4KB/partition)
                    # for full write bandwidth. The very first group uses
                    # 2/3/4 so the write stream starts ~2 chunks earlier.
                    pieces = (2, 3, 4) if (cs == 0 and b == 0) else (4, 5)
                    pc = None
                    pi = 0
                    u0 = 0
                    for u in range(N_CH):
                        po = pout.tile([128, 512], F32, space="PSUM")
                        nc.tensor.matmul(po[:],
                                         lhsT=mst_s[:, 128 * b:128 * (b + 1)],
                                         rhs=ohq[:, o2 + u * 512:o2 + u * 512 + 512],
                                         start=True, stop=True)
                        if pc is None:
                            pc = stage.tile([128, 5 * 512], F16, tag="pc")
                            u0 = u
                        uu = u - u0
                        if ci % 2 == 0:
                            nc.vector.tensor_copy(pc[:, uu * 512:uu * 512 + 512],
                                                  po[:])
                        else:
                            nc.scalar.copy(pc[:, uu * 512:uu * 512 + 512], po[:])
                        ci += 1
                        if uu == pieces[pi] - 1:
                            nsz = pieces[pi] * 512
                            nc.gpsimd.dma_start(
                                out_ap[b, :, o2 + u0 * 512:o2 + u0 * 512 + nsz],
                                pc[:, :nsz])
                            pc = None
                            pi += 1
    nc.compile()
    return nc


def _get_modules():
    if "a" not in _CACHE:
        _CACHE["a"] = _build_neff_a()
        _CACHE["b"] = _build_neff_b()
    return _CACHE["a"], _CACHE["b"]


def kernel(features, depth, weight, bias, depthpool=None):
    trace = bool(int(os.environ.get("KERNEL_TRACE", "0")))
    if trace:
        trace = _install_ntff_hook()

    features = np.asarray(features, dtype=np.float32)
    depth = np.asarray(depth, dtype=np.float32)
    weight = np.asarray(weight, dtype=np.float32)
    bias = np.asarray(bias, dtype=np.float32)

    # ---- host: histogram binning of depth (exact f32 replica of reference)
    d = depth[:, 0]                                     # [B, H, W] f32
    dmin, dmax = d.min(), d.max()
    width = np.float32((dmax - dmin) / np.float32(NB))
    bins = np.clip(np.floor((d - dmin) / width).astype(np.int32), 0, NB - 1)
    bins = bins.reshape(B, HW)
    counts = np.bincount(bins.ravel(), minlength=NB).astype(np.float64)

    # ---- per-core phase-A inputs: transposed fp16 features + bin-id map
    f16 = features.astype(np.float16)                   # [B, CIN, H, W]
    in_maps_a = []
    bins_by_core = []
    for c in range(N_CORES):
        # ftT[p, blk*CIN + cin] = feats[cin, px] for px = blk*BLK + p,
        # blk enumerated as (b, jb) to match binsT / phase-B pixel order
        fc = f16[BPC * c:BPC * (c + 1)].reshape(BPC, CIN, HW // BLK, BLK)
        ftT = np.ascontiguousarray(fc.transpose(3, 0, 2, 1)).reshape(128, -1)
        binsc = bins[BPC * c:BPC * (c + 1)].reshape(PPC)
        binsT = np.ascontiguousarray(
            binsc.reshape(N_BLOCKS, BLK).T).astype(np.float16)
        in_maps_a.append({"ft": ftT, "binsT": binsT})
        bins_by_core.append(binsc)

    nc_a, nc_b = _get_modules()
    core_ids = list(range(N_CORES))

    def _run(nc, in_maps):
        try:
            return bass_utils.run_bass_kernel_spmd(nc, in_maps,
                                                   core_ids=core_ids,
                                                   trace=trace)
        except Exception:
            # one retry for transient device hiccups
            return bass_utils.run_bass_kernel_spmd(nc, in_maps,
                                                   core_ids=core_ids,
                                                   trace=trace)

    res_a = _run(nc_a, in_maps_a)
    if trace:
        LAST_EXEC_NS["A"] = res_a.exec_time_ns
        LAST_RES["A"] = res_a

    S = np.zeros((NB, CIN), dtype=np.float64)
    for c in range(N_CORES):
        S += res_a.results[c]["spart"].astype(np.float64)

    means = (S @ weight.astype(np.float64).T) \
        / np.maximum(counts, 1.0)[:, None] \
        + bias.astype(np.float64)[None, :] * (counts > 0)[:, None]
    mh = means.astype(np.float16)                       # [NB, COUT]

    # block-diagonal stationary: rows 10g..10g+10, col block 64i hold the
    # fp16 means for quarter g = 2b+i (b = batch-in-core, i = pixel half);
    # rows 40-79 stay zero (they face the duplicated one-hot rows)
    mst = np.zeros((BPC, 80, 128), dtype=np.float16)
    for b in range(BPC):
        for i in range(2):
            g = 2 * b + i
            mst[b, 10 * g:10 * g + NB, 64 * i:64 * i + COUT] = mh

    arange_nb = np.arange(NB, dtype=np.int32)
    in_maps_b = []
    quarter = PPC // 4
    for c in range(N_CORES):
        binsc = bins_by_core[c]
        ohb = np.empty((8 * NB, quarter), dtype=np.float16)
        for g in range(4):
            ohb[10 * g:10 * g + NB] = (
                arange_nb[:, None] ==
                binsc[None, g * quarter:(g + 1) * quarter]
            ).astype(np.float16)
        ohb[40:] = ohb[:40]      # 2x replication -> 80-partition DMA rate
        in_maps_b.append({"mst": mst, "ohb": ohb})

    res_b = _run(nc_b, in_maps_b)
    if trace:
        LAST_EXEC_NS["B"] = res_b.exec_time_ns
        LAST_RES["B"] = res_b

    out = np.empty((B, COUT, H, W_), dtype=np.float32)
    for c in range(N_CORES):
        r = res_b.results[c]["out"].reshape(BPC, 2, COUT, HW // 2)
        out[BPC * c:BPC * (c + 1)] = \
            r.transpose(0, 2, 1, 3).astype(np.float32).reshape(BPC, COUT, H, W_)
    return out


# revision 9
# speedup vs baseline: 1.1347x; 1.1347x over previous
"""Trainium2 Bass kernel for nn_DepthGlobalPool (histogram_binning), v2.

Math: out[b,:,h,w] = means[bin(b,h,w)] where
  bin   = histogram bin of depth over global [min,max], 10 equal bins
  means = per-bin mean of (W @ features[p] + bias) over pixels p in the bin.
Because the 1x1 conv is linear, means = W @ (per-bin feature means) + bias,
so the conv itself runs on the HOST over the tiny [10, Cin] bin-sums and the
device only ever computes per-bin SUMS of raw features (phase A) and the
per-pixel scatter of the final means (phase B).

Distribution: data-parallel over batch B (2 batches per core on 8 cores);
the [10,128] per-core partials are reduced on host between the two NEFFs.

Phase A (per core): features arrive HOST-pre-transposed as fp16 [128px, C]
blocks (fp16 keeps the HBM read at half of f32; quantization noise averages
out over ~7M pixels per bin). Per 128-px block j one accumulating matmul
  S_ps[10, 128] += onehot_j[128px, 10].T @ ftT_j[128px, 128c]
with the one-hot as the 10-column stationary (LDWEIGHTS ~8ns, hidden) and
the feature block streamed (128 cols). The one-hot itself is built on
device from a [128, 576] fp16 bin-id map (147 KB DMA instead of 2.4 MB).

Phase B (per core): out tile [128=(half,chan), 512px] = mstat.T @ onehot
per 512-px chunk, ONE K=40 matmul per chunk: the stationary is
block-diagonal [40,128] fp16 with means at rows 10g / col block 64i for
quarter g = 2b+i; the rhs packs the 4 quarters' one-hot at rows 10g.
Output is written fp16 (means are fp16-exact through f32 PSUM) and upcast
to f32 on host during the unshard -- halves the dominant write stream.
"""

import os
import numpy as np

import concourse.bass as bass  # noqa: F401  (registers types)
import concourse.tile as tile
import concourse.bass_utils as bass_utils
from concourse import bacc, mybir

# Problem shape (hardcoded per contract)
B, CIN, COUT, H, W_ = 16, 128, 64, 192, 192
HW = H * W_                      # 36864
NB = 10                          # histogram bins
N_CORES = 8
BPC = B // N_CORES               # batches per core = 2
PPC = BPC * HW                   # pixels per core = 73728
BLK = 128                        # pixels per block
N_BLOCKS = PPC // BLK            # 576
SLAB_PX = 9216                   # pixels per feature DMA slab (2.36 MB fp16)
N_SLABS = PPC // SLAB_PX         # 8
BLK_PER_SLAB = SLAB_PX // BLK    # 72
OHA_STRIDE = 16                  # onehot block stride (pad 10->16: 32B align)

F16 = mybir.dt.float16
F32 = mybir.dt.float32

_CACHE = {}

# exec times (ns) of the last kernel() call, per NEFF, when tracing enabled
LAST_EXEC_NS = {}
LAST_RES = {}


def _install_ntff_hook():
    """Optionally enable NTFF profiling under axon (agent image lacks
    antenv.axon_hooks). Best-effort; harmless if unavailable."""
    import sys, types
    if "antenv.axon_hooks" in sys.modules:
        return True
    try:
        mod = types.ModuleType("antenv.axon_hooks")
        _hook = [None]
        mod.set_axon_ntff_profile_hook = lambda h: _hook.__setitem__(0, h)
        mod.get_axon_ntff_profile_hook = lambda: _hook[0]
        import antenv
        from trn_agent_boot.trn_boot import _ntff_profile_via_ctypes
        antenv.axon_hooks = mod
        sys.modules["antenv.axon_hooks"] = mod
        mod.set_axon_ntff_profile_hook(
            _ntff_profile_via_ctypes("/opt/axon/libaxon_pjrt.so"))
        return True
    except Exception:
        return False


def _build_neff_a():
    """Phase A: per-core per-bin sums of (transposed) features, S[10, 128]."""
    nc = bacc.Bacc("TRN2", target_bir_lowering=False, debug=False,
                   enable_asserts=True, num_devices=N_CORES)
    ft_t = nc.dram_tensor("ft", [128, PPC], F16, kind="ExternalInput")
    bins_t = nc.dram_tensor("binsT", [128, N_BLOCKS], F16, kind="ExternalInput")
    spart_t = nc.dram_tensor("spart", [NB, CIN], F32, kind="ExternalOutput")

    ft = ft_t.ap()
    with tile.TileContext(nc) as tc:
        with tc.tile_pool(name="cst", bufs=1) as cst, \
             tc.tile_pool(name="fpool", bufs=4) as fpool, \
             tc.tile_pool(name="spool", bufs=1) as spool, \
             tc.tile_pool(name="pwarm", bufs=1, space="PSUM") as pwarm, \
             tc.tile_pool(name="pacc", bufs=1, space="PSUM") as pacc:

            binsT = cst.tile([128, N_BLOCKS], F16)
            nc.scalar.dma_start(binsT[:], bins_t.ap()[:])

            # dependency-free warmup burst: dense matmuls trip the PE HAM
            # clock-gate to 2.4 GHz while the first DMAs land; long enough
            # that the PE never sees a >3.4us idle gap before real work
            warm = cst.tile([128, 512], F16)
            nc.gpsimd.memset(warm[:], 0)
            wps = pwarm.tile([128, 512], F32, space="PSUM")
            for _ in range(20):
                nc.tensor.matmul(wps[:], lhsT=warm[:, :128], rhs=warm[:],
                                 start=True, stop=True)

            # one-hot oha[p, j, n] = (binsT[p, j] == n), built on device.
            # Two column-chunks so the first slabs' matmuls don't wait on
            # the whole [128, 576] sweep.
            oha = cst.tile([128, N_BLOCKS, OHA_STRIDE], F16)
            CHUNK = [(0, 3 * BLK_PER_SLAB), (3 * BLK_PER_SLAB, N_BLOCKS)]
            for j0, j1 in CHUNK:
                for n in range(NB):
                    nc.vector.tensor_scalar(
                        oha[:, j0:j1, n], binsT[:, j0:j1], float(n), None,
                        op0=mybir.AluOpType.is_equal)

            # 4-way column-tiled accumulators: block j accumulates into PE
            # column strip 32*(j%4), so 4 matmuls are in flight in the
            # array at once (the 10-col stationaries only need a strip)
            S_ps = pacc.tile([128, CIN], F32, space="PSUM")
            HALF = SLAB_PX // 2
            for s in range(N_SLABS):
                fs = fpool.tile([128, SLAB_PX], F16)
                # each slab's two 1.18MB halves ride DIFFERENT HWDGE rings
                # (sync + scalar): slab arrival order == program order even
                # when the rings drift, so the FIFO PE never waits on an
                # out-of-order slab while a later one sits ready
                nc.sync.dma_start(fs[:, 0:HALF], ft[:, s * SLAB_PX:
                                                     s * SLAB_PX + HALF])
                nc.scalar.dma_start(fs[:, HALF:], ft[:, s * SLAB_PX + HALF:
                                                     (s + 1) * SLAB_PX])
                for j in range(BLK_PER_SLAB):
                    blk = s * BLK_PER_SLAB + j
                    q = 32 * (blk % 4)
                    nc.tensor.matmul(
                        S_ps[q:q + NB, :],
                        lhsT=oha[:, blk, 0:NB],
                        rhs=fs[:, j * BLK:(j + 1) * BLK],
                        start=(blk < 4), stop=(blk >= N_BLOCKS - 4),
                        tile_position=(0, q))

            # PSUM-reduce the 4 strips (ops may read at most ONE PSUM input)
            s_acc = spool.tile([NB, CIN], F32)
            nc.vector.tensor_copy(s_acc[:], S_ps[0:NB, :])
            for q in (32, 64, 96):
                nc.vector.tensor_add(s_acc[:], s_acc[:], S_ps[q:q + NB, :])
            nc.sync.dma_start(spart_t.ap()[:], s_acc[:])
    nc.compile()
    return nc


def _build_neff_b():
    """Phase B: out[b,:,p] = means[bin(p)] via a means-stationary matmul,
    with TWO pixels packed per f32 output element (fixed-point trick).

    The pacer of a naive scatter is the forced PSUM->SBUF copy (DVE/ACT
    time scales with free-axis elements, ~0.68us per [128,512] chunk). So:
    quantize means to 11-bit fixed point, m_off = round(m/s)+1024 in
    [0,2047] -- EXACT in fp16 -- and give each chunk PAIR-columns: the
    even pixel's one-hot rows (10g+n, quarter g=2b+i) carry 1.0 and the
    odd pixel's rows (40+10g+n) carry 2048.0, both facing stationary
    value m_off. PSUM then holds the exact integer
    m_off_even + 2048*m_off_odd < 2^22: one f32 per TWO pixels -> half
    the matmuls, half the copy elements, same write bytes. The host
    unpacks with shift/mask during the unshard.
    """
    nc = bacc.Bacc("TRN2", target_bir_lowering=False, debug=False,
                   enable_asserts=True, num_devices=N_CORES)
    NPAIR = PPC // 8             # pair-columns per quarter = 9216
    mst_t = nc.dram_tensor("mst", [BPC, 80, 128], F16, kind="ExternalInput")
    ohb_t = nc.dram_tensor("ohb", [80, NPAIR], F16, kind="ExternalInput")
    out_t = nc.dram_tensor("out", [BPC, 128, NPAIR], F32, kind="ExternalOutput")

    SLAB = 4608                  # pair-columns per slab
    N_CH = SLAB // 512           # 9 psum chunks per slab

    out_ap = out_t.ap()
    ohb = ohb_t.ap()
    with tile.TileContext(nc) as tc:
        with tc.tile_pool(name="cst", bufs=1) as cst, \
             tc.tile_pool(name="stage", bufs=6) as stage, \
             tc.tile_pool(name="pwarm", bufs=1, space="PSUM") as pwarm, \
             tc.tile_pool(name="pout", bufs=7, space="PSUM") as pout:

            mst_s = cst.tile([80, BPC * 128], F16)
            for b in range(BPC):
                nc.sync.dma_start(mst_s[:, 128 * b:128 * (b + 1)],
                                  mst_t.ap()[b])

            # whole one-hot in ONE persistent tile: [80, *] reaches ~10 of
            # 16 SDMA engines (~270 GB/s) vs ~5 for [40, *]; 8 up-front
            # piece-DMAs so the first chunks start after ~1 piece
            ohq = cst.tile([80, NPAIR], F16)
            QD = NPAIR // 8
            for q in range(8):
                nc.sync.dma_start(ohq[:, q * QD:(q + 1) * QD],
                                  ohb[:, q * QD:(q + 1) * QD])

            # warmup burst for the PE HAM clock-gate (overlaps input DMAs)
            warm = cst.tile([128, 512], F16)
            nc.gpsimd.memset(warm[:], 0)
            # prime the SWDGE write path so the first real output write
            # doesn't pay the descriptor-ring bootstrap
            scr_t = nc.dram_tensor("scratch", [128, 64], F16, kind="Internal")
            nc.gpsimd.dma_start(scr_t.ap()[:], warm[:, 0:64])
            wps = pwarm.tile([128, 512], F32, space="PSUM")
            for _ in range(16):
                nc.tensor.matmul(wps[:], lhsT=warm[:, :128], rhs=warm[:],
                                 start=True, stop=True)

            ci = 0
            for cs in range(2):      # pair-column slab: cols [o2, o2+4608)
                o2 = cs * SLAB
                for b in range(BPC):
                    # 9 chunks staged as pieces of 4/5 chunks: ~1MB write
                    # DMAs with 8-10KB/partition descriptor lines. The very
                    # first group uses 2/3/4 to start the write stream early.
                    pieces = (2, 3, 4) if (cs == 0 and b == 0) else (4, 5)
                    pc = None
                    pi = 0
                    u0 = 0
                    for u in range(N_CH):
                        po = pout.tile([128, 512], F32, space="PSUM")
                        nc.tensor.matmul(po[:],
                                         lhsT=mst_s[:, 128 * b:128 * (b + 1)],
                                         rhs=ohq[:, o2 + u * 512:o2 + u * 512 + 512],
                                         start=True, stop=True)
                        if pc is None:
                            pc = stage.tile([128, 5 * 512], F32, tag="pc")
                            u0 = u
                        uu = u - u0
                        if ci % 2 == 0:
                            nc.vector.tensor_copy(pc[:, uu * 512:uu * 512 + 512],
                                                  po[:])
                        else:
                            nc.scalar.copy(pc[:, uu * 512:uu * 512 + 512], po[:])
                        ci += 1
                        if uu == pieces[pi] - 1:
                            nsz = pieces[pi] * 512
                            nc.gpsimd.dma_start(
                                out_ap[b, :, o2 + u0 * 512:o2 + u0 * 512 + nsz],
                                pc[:, :nsz])
                            pc = None
                            pi += 1
    nc.compile()
    return nc


def _get_modules():
    if "a" not in _CACHE:
        _CACHE["a"] = _build_neff_a()
        _CACHE["b"] = _build_neff_b()
    return _CACHE["a"], _CACHE["b"]


def kernel(features, depth, weight, bias, depthpool=None):
    trace = bool(int(os.environ.get("KERNEL_TRACE", "0")))
    if trace:
        trace = _install_ntff_hook()

    features = np.asarray(features, dtype=np.float32)
    depth = np.asarray(depth, dtype=np.float32)
    weight = np.asarray(weight, dtype=np.float32)
    bias = np.asarray(bias, dtype=np.float32)

    # ---- host: histogram binning of depth (exact f32 replica of reference)
    d = depth[:, 0]                                     # [B, H, W] f32
    dmin, dmax = d.min(), d.max()
    width = np.float32((dmax - dmin) / np.float32(NB))
    bins = np.clip(np.floor((d - dmin) / width).astype(np.int32), 0, NB - 1)
    bins = bins.reshape(B, HW)
    counts = np.bincount(bins.ravel(), minlength=NB).astype(np.float64)

    # ---- per-core phase-A inputs: transposed fp16 features + bin-id map
    f16 = features.astype(np.float16)                   # [B, CIN, H, W]
    in_maps_a = []
    bins_by_core = []
    for c in range(N_CORES):
        # ftT[p, blk*CIN + cin] = feats[cin, px] for px = blk*BLK + p,
        # blk enumerated as (b, jb) to match binsT / phase-B pixel order
        fc = f16[BPC * c:BPC * (c + 1)].reshape(BPC, CIN, HW // BLK, BLK)
        ftT = np.ascontiguousarray(fc.transpose(3, 0, 2, 1)).reshape(128, -1)
        binsc = bins[BPC * c:BPC * (c + 1)].reshape(PPC)
        binsT = np.ascontiguousarray(
            binsc.reshape(N_BLOCKS, BLK).T).astype(np.float16)
        in_maps_a.append({"ft": ftT, "binsT": binsT})
        bins_by_core.append(binsc)

    nc_a, nc_b = _get_modules()
    core_ids = list(range(N_CORES))

    def _run(nc, in_maps):
        try:
            return bass_utils.run_bass_kernel_spmd(nc, in_maps,
                                                   core_ids=core_ids,
                                                   trace=trace)
        except Exception:
            # one retry for transient device hiccups
            return bass_utils.run_bass_kernel_spmd(nc, in_maps,
                                                   core_ids=core_ids,
                                                   trace=trace)

    res_a = _run(nc_a, in_maps_a)
    if trace:
        LAST_EXEC_NS["A"] = res_a.exec_time_ns
        LAST_RES["A"] = res_a

    S = np.zeros((NB, CIN), dtype=np.float64)
    for c in range(N_CORES):
        S += res_a.results[c]["spart"].astype(np.float64)

    means = (S @ weight.astype(np.float64).T) \
        / np.maximum(counts, 1.0)[:, None] \
        + bias.astype(np.float64)[None, :] * (counts > 0)[:, None]
    # 11-bit fixed-point means: m_off = round(m/s)+1024 in [0,2047] is an
    # EXACT fp16 integer, so the PE's m_off*1 + m_off*2048 products and
    # their f32 sum (< 2^22) are exact -- two pixels per f32 element
    s_q = max(float(np.abs(means).max()) / 1023.0, 1e-30)
    mq_off = (np.clip(np.round(means / s_q), -1024, 1023) + 1024.0) \
        .astype(np.float16)                                  # [NB, COUT]

    # stationary: rows 10g..10g+10 (even px) and 40+10g..+10 (odd px) of
    # quarter g = 2b+i both hold m_off at col block 64i
    mst = np.zeros((BPC, 80, 128), dtype=np.float16)
    for b in range(BPC):
        for i in range(2):
            g = 2 * b + i
            mst[b, 10 * g:10 * g + NB, 64 * i:64 * i + COUT] = mq_off
            mst[b, 40 + 10 * g:40 + 10 * g + NB, 64 * i:64 * i + COUT] = mq_off

    arange_nb = np.arange(NB, dtype=np.int32)
    in_maps_b = []
    quarter = PPC // 4
    npair = quarter // 2
    for c in range(N_CORES):
        binsc = bins_by_core[c]
        ohb = np.empty((80, npair), dtype=np.float16)
        for g in range(4):
            q = binsc[g * quarter:(g + 1) * quarter]
            ohb[10 * g:10 * g + NB] = (
                arange_nb[:, None] == q[None, 0::2]).astype(np.float16)
            ohb[40 + 10 * g:40 + 10 * g + NB] = np.float16(2048.0) * (
                arange_nb[:, None] == q[None, 1::2]).astype(np.float16)
        in_maps_b.append({"mst": mst, "ohb": ohb})

    res_b = _run(nc_b, in_maps_b)
    if trace:
        LAST_EXEC_NS["B"] = res_b.exec_time_ns
        LAST_RES["B"] = res_b

    # unpack: v = m_off_even + 2048*m_off_odd (exact integer in f32)
    out = np.empty((B, COUT, H, W_), dtype=np.float32)
    sf = np.float32(s_q)
    for c in range(N_CORES):
        vi = res_b.results[c]["out"].astype(np.int32)        # [BPC, 128, 9216]
        vi = vi.reshape(BPC, 2, COUT, npair)
        tmp = np.empty((BPC, 2, COUT, npair, 2), dtype=np.float32)
        tmp[..., 0] = (vi & 2047) - 1024
        tmp[..., 1] = (vi >> 11) - 1024
        oc = tmp.transpose(0, 2, 1, 3, 4).reshape(BPC, COUT, HW) * sf
        out[BPC * c:BPC * (c + 1)] = oc.reshape(BPC, COUT, H, W_)
    return out


# revision 10
# speedup vs baseline: 1.1861x; 1.0453x over previous
"""Trainium2 Bass kernel for nn_DepthGlobalPool (histogram_binning), v2.

Math: out[b,:,h,w] = means[bin(b,h,w)] where
  bin   = histogram bin of depth over global [min,max], 10 equal bins
  means = per-bin mean of (W @ features[p] + bias) over pixels p in the bin.
Because the 1x1 conv is linear, means = W @ (per-bin feature means) + bias,
so the conv itself runs on the HOST over the tiny [10, Cin] bin-sums and the
device only ever computes per-bin SUMS of raw features (phase A) and the
per-pixel scatter of the final means (phase B).

Distribution: data-parallel over batch B (2 batches per core on 8 cores);
the [10,128] per-core partials are reduced on host between the two NEFFs.

Phase A (per core): features arrive HOST-pre-transposed as fp16 [128px, C]
blocks (fp16 keeps the HBM read at half of f32; quantization noise averages
out over ~7M pixels per bin). Per 128-px block j one accumulating matmul
  S_ps[10, 128] += onehot_j[128px, 10].T @ ftT_j[128px, 128c]
with the one-hot as the 10-column stationary (LDWEIGHTS ~8ns, hidden) and
the feature block streamed (128 cols). The one-hot itself is built on
device from a [128, 576] fp16 bin-id map (147 KB DMA instead of 2.4 MB).

Phase B (per core): out tile [128=(half,chan), 512px] = mstat.T @ onehot
per 512-px chunk, ONE K=40 matmul per chunk: the stationary is
block-diagonal [40,128] fp16 with means at rows 10g / col block 64i for
quarter g = 2b+i; the rhs packs the 4 quarters' one-hot at rows 10g.
Output is written fp16 (means are fp16-exact through f32 PSUM) and upcast
to f32 on host during the unshard -- halves the dominant write stream.
"""

import os
import numpy as np

import concourse.bass as bass  # noqa: F401  (registers types)
import concourse.tile as tile
import concourse.bass_utils as bass_utils
from concourse import bacc, mybir

# Problem shape (hardcoded per contract)
B, CIN, COUT, H, W_ = 16, 128, 64, 192, 192
HW = H * W_                      # 36864
NB = 10                          # histogram bins
N_CORES = 8
BPC = B // N_CORES               # batches per core = 2
PPC = BPC * HW                   # pixels per core = 73728
BLK = 128                        # pixels per block
N_BLOCKS = PPC // BLK            # 576
SLAB_PX = 9216                   # pixels per feature DMA slab (2.36 MB fp16)
N_SLABS = PPC // SLAB_PX         # 8
BLK_PER_SLAB = SLAB_PX // BLK    # 72
OHA_STRIDE = 16                  # onehot block stride (pad 10->16: 32B align)

F16 = mybir.dt.float16
F32 = mybir.dt.float32

_CACHE = {}

# exec times (ns) of the last kernel() call, per NEFF, when tracing enabled
LAST_EXEC_NS = {}
LAST_RES = {}


def _install_ntff_hook():
    """Optionally enable NTFF profiling under axon (agent image lacks
    antenv.axon_hooks). Best-effort; harmless if unavailable."""
    import sys, types
    if "antenv.axon_hooks" in sys.modules:
        return True
    try:
        mod = types.ModuleType("antenv.axon_hooks")
        _hook = [None]
        mod.set_axon_ntff_profile_hook = lambda h: _hook.__setitem__(0, h)
        mod.get_axon_ntff_profile_hook = lambda: _hook[0]
        import antenv
        from trn_agent_boot.trn_boot import _ntff_profile_via_ctypes
        antenv.axon_hooks = mod
        sys.modules["antenv.axon_hooks"] = mod
        mod.set_axon_ntff_profile_hook(
            _ntff_profile_via_ctypes("/opt/axon/libaxon_pjrt.so"))
        return True
    except Exception:
        return False


def _build_neff_a():
    """Phase A: per-core per-bin sums of (transposed) features, S[10, 128]."""
    nc = bacc.Bacc("TRN2", target_bir_lowering=False, debug=False,
                   enable_asserts=True, num_devices=N_CORES)
    ft_t = nc.dram_tensor("ft", [128, PPC], F16, kind="ExternalInput")
    bins_t = nc.dram_tensor("binsT", [128, N_BLOCKS], F16, kind="ExternalInput")
    spart_t = nc.dram_tensor("spart", [NB, CIN], F32, kind="ExternalOutput")

    ft = ft_t.ap()
    with tile.TileContext(nc) as tc:
        with tc.tile_pool(name="cst", bufs=1) as cst, \
             tc.tile_pool(name="fpool", bufs=4) as fpool, \
             tc.tile_pool(name="spool", bufs=1) as spool, \
             tc.tile_pool(name="pwarm", bufs=1, space="PSUM") as pwarm, \
             tc.tile_pool(name="pacc", bufs=1, space="PSUM") as pacc:

            binsT = cst.tile([128, N_BLOCKS], F16)
            nc.scalar.dma_start(binsT[:], bins_t.ap()[:])

            # dependency-free warmup burst: dense matmuls trip the PE HAM
            # clock-gate to 2.4 GHz while the first DMAs land; long enough
            # that the PE never sees a >3.4us idle gap before real work
            warm = cst.tile([128, 512], F16)
            nc.gpsimd.memset(warm[:], 0)
            wps = pwarm.tile([128, 512], F32, space="PSUM")
            for _ in range(20):
                nc.tensor.matmul(wps[:], lhsT=warm[:, :128], rhs=warm[:],
                                 start=True, stop=True)

            # one-hot oha[p, j, n] = (binsT[p, j] == n), built on device.
            # Two column-chunks so the first slabs' matmuls don't wait on
            # the whole [128, 576] sweep.
            oha = cst.tile([128, N_BLOCKS, OHA_STRIDE], F16)
            CHUNK = [(0, 3 * BLK_PER_SLAB), (3 * BLK_PER_SLAB, N_BLOCKS)]
            for j0, j1 in CHUNK:
                for n in range(NB):
                    nc.vector.tensor_scalar(
                        oha[:, j0:j1, n], binsT[:, j0:j1], float(n), None,
                        op0=mybir.AluOpType.is_equal)

            # 4-way column-tiled accumulators: block j accumulates into PE
            # column strip 32*(j%4), so 4 matmuls are in flight in the
            # array at once (the 10-col stationaries only need a strip)
            S_ps = pacc.tile([128, CIN], F32, space="PSUM")
            HALF = SLAB_PX // 2
            for s in range(N_SLABS):
                fs = fpool.tile([128, SLAB_PX], F16)
                # each slab's two 1.18MB halves ride DIFFERENT HWDGE rings
                # (sync + scalar): slab arrival order == program order even
                # when the rings drift, so the FIFO PE never waits on an
                # out-of-order slab while a later one sits ready
                nc.sync.dma_start(fs[:, 0:HALF], ft[:, s * SLAB_PX:
                                                     s * SLAB_PX + HALF])
                nc.scalar.dma_start(fs[:, HALF:], ft[:, s * SLAB_PX + HALF:
                                                     (s + 1) * SLAB_PX])
                for j in range(BLK_PER_SLAB):
                    blk = s * BLK_PER_SLAB + j
                    q = 32 * (blk % 4)
                    nc.tensor.matmul(
                        S_ps[q:q + NB, :],
                        lhsT=oha[:, blk, 0:NB],
                        rhs=fs[:, j * BLK:(j + 1) * BLK],
                        start=(blk < 4), stop=(blk >= N_BLOCKS - 4),
                        tile_position=(0, q))

            # PSUM-reduce the 4 strips (ops may read at most ONE PSUM input)
            s_acc = spool.tile([NB, CIN], F32)
            nc.vector.tensor_copy(s_acc[:], S_ps[0:NB, :])
            for q in (32, 64, 96):
                nc.vector.tensor_add(s_acc[:], s_acc[:], S_ps[q:q + NB, :])
            nc.sync.dma_start(spart_t.ap()[:], s_acc[:])
    nc.compile()
    return nc


def _build_neff_b():
    """Phase B: out[b,:,p] = means[bin(p)] via a means-stationary matmul,
    with THREE pixels packed per f32 output element (fixed-point trick).

    The pacers of a naive scatter are the forced PSUM->SBUF copy (DVE/ACT
    time scales with free-axis elements) and the HBM write stream. So:
    quantize means to 8-bit fixed point per channel, m_off = round(m/s_c)
    + 128 in [0,255] -- EXACT in fp16 -- and give each [128, 512] chunk
    TRIPLE-columns: pixel 3t of quarter g=2b+i drives one-hot rows 10g+n
    with value 1.0, pixel 3t+1 rows 40+10g+n with 256.0, pixel 3t+2 rows
    80+10g+n with 65536.0, all facing stationary value m_off. PSUM then
    holds the exact integer q0 + 256*q1 + 65536*q2 < 2^24: one f32 per
    THREE pixels -> 1/3 the matmuls, copies and write bytes. The host
    unpacks with shift/mask during the unshard.
    """
    nc = bacc.Bacc("TRN2", target_bir_lowering=False, debug=False,
                   enable_asserts=True, num_devices=N_CORES)
    NTRIP = (PPC // 4) // 3      # triple-columns per quarter = 6144
    mst_t = nc.dram_tensor("mst", [BPC, 120, 128], F16, kind="ExternalInput")
    ohb_t = nc.dram_tensor("ohb", [120, NTRIP], F16, kind="ExternalInput")
    out_t = nc.dram_tensor("out", [BPC, 128, NTRIP], F32, kind="ExternalOutput")

    N_CH = NTRIP // 512          # 12 psum chunks per batch

    out_ap = out_t.ap()
    ohb = ohb_t.ap()
    with tile.TileContext(nc) as tc:
        with tc.tile_pool(name="cst", bufs=1) as cst, \
             tc.tile_pool(name="stage", bufs=6) as stage, \
             tc.tile_pool(name="pwarm", bufs=1, space="PSUM") as pwarm, \
             tc.tile_pool(name="pout", bufs=7, space="PSUM") as pout:

            mst_s = cst.tile([120, BPC * 128], F16)
            for b in range(BPC):
                nc.sync.dma_start(mst_s[:, 128 * b:128 * (b + 1)],
                                  mst_t.ap()[b])

            # whole one-hot in ONE persistent tile: [120, *] reaches ~15 of
            # 16 SDMA engines; 8 up-front piece-DMAs so the first chunks
            # start after ~1 piece
            ohq = cst.tile([120, NTRIP], F16)
            QD = NTRIP // 8
            for q in range(8):
                nc.sync.dma_start(ohq[:, q * QD:(q + 1) * QD],
                                  ohb[:, q * QD:(q + 1) * QD])

            # warmup burst for the PE HAM clock-gate (overlaps input DMAs)
            warm = cst.tile([128, 512], F16)
            nc.gpsimd.memset(warm[:], 0)
            # prime the SWDGE write path so the first real output write
            # doesn't pay the descriptor-ring bootstrap
            scr_t = nc.dram_tensor("scratch", [128, 64], F16, kind="Internal")
            nc.gpsimd.dma_start(scr_t.ap()[:], warm[:, 0:64])
            wps = pwarm.tile([128, 512], F32, space="PSUM")
            for _ in range(16):
                nc.tensor.matmul(wps[:], lhsT=warm[:, :128], rhs=warm[:],
                                 start=True, stop=True)

            ci = 0
            for b in range(BPC):
                # 12 chunks staged as pieces of ~1MB write DMAs with fat
                # (8KB/partition) descriptor lines; the very first pieces
                # are small to start the write stream early
                pieces = (2, 3, 3, 4) if b == 0 else (4, 4, 4)
                pc = None
                pi = 0
                u0 = 0
                for u in range(N_CH):
                    po = pout.tile([128, 512], F32, space="PSUM")
                    nc.tensor.matmul(po[:],
                                     lhsT=mst_s[:, 128 * b:128 * (b + 1)],
                                     rhs=ohq[:, u * 512:u * 512 + 512],
                                     start=True, stop=True)
                    if pc is None:
                        pc = stage.tile([128, 4 * 512], F32, tag="pc")
                        u0 = u
                    uu = u - u0
                    if ci % 2 == 0:
                        nc.vector.tensor_copy(pc[:, uu * 512:uu * 512 + 512],
                                              po[:])
                    else:
                        nc.scalar.copy(pc[:, uu * 512:uu * 512 + 512], po[:])
                    ci += 1
                    if uu == pieces[pi] - 1:
                        nsz = pieces[pi] * 512
                        nc.gpsimd.dma_start(
                            out_ap[b, :, u0 * 512:u0 * 512 + nsz],
                            pc[:, :nsz])
                        pc = None
                        pi += 1
    nc.compile()
    return nc


def _get_modules():
    if "a" not in _CACHE:
        _CACHE["a"] = _build_neff_a()
        _CACHE["b"] = _build_neff_b()
    return _CACHE["a"], _CACHE["b"]


def kernel(features, depth, weight, bias, depthpool=None):
    trace = bool(int(os.environ.get("KERNEL_TRACE", "0")))
    if trace:
        trace = _install_ntff_hook()

    features = np.asarray(features, dtype=np.float32)
    depth = np.asarray(depth, dtype=np.float32)
    weight = np.asarray(weight, dtype=np.float32)
    bias = np.asarray(bias, dtype=np.float32)

    # ---- host: histogram binning of depth (exact f32 replica of reference)
    d = depth[:, 0]                                     # [B, H, W] f32
    dmin, dmax = d.min(), d.max()
    width = np.float32((dmax - dmin) / np.float32(NB))
    bins = np.clip(np.floor((d - dmin) / width).astype(np.int32), 0, NB - 1)
    bins = bins.reshape(B, HW)
    counts = np.bincount(bins.ravel(), minlength=NB).astype(np.float64)

    # ---- per-core phase-A inputs: transposed fp16 features + bin-id map
    f16 = features.astype(np.float16)                   # [B, CIN, H, W]
    in_maps_a = []
    bins_by_core = []
    for c in range(N_CORES):
        # ftT[p, blk*CIN + cin] = feats[cin, px] for px = blk*BLK + p,
        # blk enumerated as (b, jb) to match binsT / phase-B pixel order
        fc = f16[BPC * c:BPC * (c + 1)].reshape(BPC, CIN, HW // BLK, BLK)
        ftT = np.ascontiguousarray(fc.transpose(3, 0, 2, 1)).reshape(128, -1)
        binsc = bins[BPC * c:BPC * (c + 1)].reshape(PPC)
        binsT = np.ascontiguousarray(
            binsc.reshape(N_BLOCKS, BLK).T).astype(np.float16)
        in_maps_a.append({"ft": ftT, "binsT": binsT})
        bins_by_core.append(binsc)

    nc_a, nc_b = _get_modules()
    core_ids = list(range(N_CORES))

    def _run(nc, in_maps):
        try:
            return bass_utils.run_bass_kernel_spmd(nc, in_maps,
                                                   core_ids=core_ids,
                                                   trace=trace)
        except Exception:
            # one retry for transient device hiccups
            return bass_utils.run_bass_kernel_spmd(nc, in_maps,
                                                   core_ids=core_ids,
                                                   trace=trace)

    res_a = _run(nc_a, in_maps_a)
    if trace:
        LAST_EXEC_NS["A"] = res_a.exec_time_ns
        LAST_RES["A"] = res_a

    S = np.zeros((NB, CIN), dtype=np.float64)
    for c in range(N_CORES):
        S += res_a.results[c]["spart"].astype(np.float64)

    means = (S @ weight.astype(np.float64).T) \
        / np.maximum(counts, 1.0)[:, None] \
        + bias.astype(np.float64)[None, :] * (counts > 0)[:, None]
    # 8-bit fixed-point means, per-channel scale: m_off = round(m/s_c)+128
    # in [0,255] is an EXACT fp16 integer, so the PE's m_off * {1, 256,
    # 65536} products and their f32 sum (< 2^24) are exact -- THREE pixels
    # per f32 element
    s_vec = np.maximum(np.abs(means).max(axis=0) / 127.5, 1e-30)  # [COUT]
    mq_off = (np.clip(np.round(means / s_vec[None, :]), -128, 127) + 128.0) \
        .astype(np.float16)                                  # [NB, COUT]

    # stationary: rows k*40 + 10g .. +10 (pixel 3t+k) of quarter g = 2b+i
    # at col block 64i. The k=2 field's 2^16 shift exceeds fp16 range
    # (max 65504), so it is SPLIT: stationary holds m_off*256 (exact, 8
    # significant bits) and the one-hot row holds 256.0.
    st_vals = (mq_off.astype(np.float32),
               mq_off.astype(np.float32),
               mq_off.astype(np.float32) * 256.0)
    mst = np.zeros((BPC, 120, 128), dtype=np.float16)
    for b in range(BPC):
        for i in range(2):
            g = 2 * b + i
            for k in range(3):
                mst[b, 40 * k + 10 * g:40 * k + 10 * g + NB,
                    64 * i:64 * i + COUT] = st_vals[k]

    arange_nb = np.arange(NB, dtype=np.int32)
    in_maps_b = []
    quarter = PPC // 4
    ntrip = quarter // 3
    factors = (np.float16(1.0), np.float16(256.0), np.float16(256.0))
    for c in range(N_CORES):
        binsc = bins_by_core[c]
        ohb = np.empty((120, ntrip), dtype=np.float16)
        for g in range(4):
            q = binsc[g * quarter:(g + 1) * quarter]
            for k in range(3):
                ohb[40 * k + 10 * g:40 * k + 10 * g + NB] = factors[k] * (
                    arange_nb[:, None] == q[None, k::3]).astype(np.float16)
        in_maps_b.append({"mst": mst, "ohb": ohb})

    res_b = _run(nc_b, in_maps_b)
    if trace:
        LAST_EXEC_NS["B"] = res_b.exec_time_ns
        LAST_RES["B"] = res_b

    # unpack: v = q0 + 256*q1 + 65536*q2 (exact integer in f32)
    out = np.empty((B, COUT, H, W_), dtype=np.float32)
    sf = s_vec.astype(np.float32).reshape(1, COUT, 1)
    for c in range(N_CORES):
        vi = res_b.results[c]["out"].astype(np.int32)        # [BPC, 128, 6144]
        vi = vi.reshape(BPC, 2, COUT, ntrip)
        tmp = np.empty((BPC, 2, COUT, ntrip, 3), dtype=np.float32)
        tmp[..., 0] = (vi & 255) - 128
        tmp[..., 1] = ((vi >> 8) & 255) - 128
        tmp[..., 2] = (vi >> 16) - 128
        oc = tmp.transpose(0, 2, 1, 3, 4).reshape(BPC, COUT, HW) * sf
        out[BPC * c:BPC * (c + 1)] = oc.reshape(BPC, COUT, H, W_)
    return out
